# revision 12
# baseline (speedup 1.0000x reference)
"""Trainium2 Bass kernel for the BMoIE (dense mixture-of-experts) network.

Network (per sample):
    alpha = softmax(x @ gate_w + gate_b)                       # [B, 8]
    h = relu(sum_k alpha_k * (h @ w_l[k] + b_l[k]))            # 3 hidden blocks
    out = sum_k alpha_k * (h @ wo[k] + bo[k])                  # output block

Strategy: data-parallel over 8 NeuronCores (2048 rows each, SPMD, no
collectives). On each core, activations are kept feature-major ("hT",
[128 feat-partitions x batch]) so they can be the stationary matmul
operand; expert weights stream in natural layout as the moving operand,
producing batch-major per-expert outputs in PSUM. Matmul operands are
bf16 (full PE speed, half the HBM traffic of fp32; rel-l2 ~5e-3 vs the
2e-2 budget). The alpha-weighted sum over the 8 experts is split across
engines: 5 experts as a VectorE scalar_tensor_tensor chain (fp32
accumulate), 3 experts scaled on ScalarE with partial sums on GpSimd.
The combined result is written as bf16 (the next layer's matmul input
is bf16 anyway), so the PE re-transposes to feature-major run at 1
cycle/row. Weights load as one DMA per (layer, expert) for few, large
transfers.
"""

import sys

sys.path.insert(0, "/opt/trn_rl_repo")

import numpy as np

import concourse.bass as bass
import concourse.mybir as mybir
import concourse.tile as tile
from concourse import bacc
from concourse.bass_utils import run_bass_kernel_spmd
from concourse.masks import make_identity

P = 128           # partitions
D = 512           # model dim (= hidden dim)
K = 8             # experts
NCORES = 8
B = 16384
R = B // NCORES   # rows per core
NT = R // P       # 16 batch tiles per core
NCH = D // P      # 4 feature chunks
F32 = mybir.dt.float32
FR = mybir.dt.float32r
BF16 = mybir.dt.bfloat16
AF = mybir.ActivationFunctionType
ALU = mybir.AluOpType

W_NAMES = ("w0", "w1", "w2", "wo")
B_NAMES = ("b0", "b1", "b2", "bo")


def _build(has_gate_b, has_bias, w_bufs=None, ht_bufs=None, z_bufs=7, repeat=1,
           mode="full", gp_offload=3, bf16=False, split=True):
    """Trace + compile the per-core kernel. has_bias is a 4-tuple of bools.

    repeat>1 runs the whole 4-layer stack that many times (same weights,
    full DMA traffic each time) — used only for timing measurements.
    mode: "full" | "pe_only" (no combine/evict) — ablation variants for
    timing only (wrong results except "full").
    gp_offload: number of experts whose scale runs on ScalarE with the
    partial sum on GpSimd (reduces VectorE op count).
    """
    from contextlib import ExitStack

    MD = BF16 if bf16 else FR  # matmul operand dtype
    SD = MD                    # stationary (activations) dtype
    if w_bufs is None:
        w_bufs = 20 if bf16 else 12
    if ht_bufs is None:
        ht_bufs = 24 if bf16 else 20

    nc = bacc.Bacc("TRN2", target_bir_lowering=False, num_devices=NCORES)
    x = nc.dram_tensor("x", [R, D], F32, kind="ExternalInput")
    gate_w = nc.dram_tensor("gate_w", [D, K], MD, kind="ExternalInput")
    gate_b = nc.dram_tensor("gate_b", [K], F32, kind="ExternalInput")
    ws = [nc.dram_tensor(n, [K, D, D], MD, kind="ExternalInput") for n in W_NAMES]
    bs = [nc.dram_tensor(n, [K, D], FR, kind="ExternalInput") for n in B_NAMES]
    out = nc.dram_tensor("out", [R, D], F32, kind="ExternalOutput")

    any_bias = any(has_bias)

    with tile.TileContext(nc) as tc, ExitStack() as ctx:
        cst = ctx.enter_context(tc.tile_pool(name="cst", bufs=1))
        wpool = ctx.enter_context(tc.tile_pool(name="wpool", bufs=w_bufs))
        htp = ctx.enter_context(tc.tile_pool(name="htp", bufs=ht_bufs))
        accp = ctx.enter_context(tc.tile_pool(name="accp", bufs=3))
        accbp = ctx.enter_context(tc.tile_pool(name="accbp", bufs=3))
        xbp = ctx.enter_context(tc.tile_pool(name="xbp", bufs=3))
        smp = ctx.enter_context(tc.tile_pool(name="smp", bufs=4))
        zp = ctx.enter_context(tc.tile_pool(name="zp", bufs=z_bufs, space="PSUM"))
        trp = ctx.enter_context(tc.tile_pool(name="trp", bufs=8 - z_bufs, space="PSUM"))

        ident = cst.tile([P, P], F32, tag="ident")
        make_identity(nc, ident[:])
        identb = cst.tile([P, P], SD, tag="identb")
        nc.scalar.activation(identb[:], ident[:], AF.Copy)

        # gate_w [512, 8] -> [128, 4*8] (chunk c at cols c*8..)
        gw = cst.tile([P, NCH * K], MD, tag="gw")
        for c in range(NCH):
            nc.sync.dma_start(gw[:, c * K:(c + 1) * K], gate_w[c * P:(c + 1) * P, :])

        gb_bc = None
        if has_gate_b:
            ones_row = cst.tile([1, P], F32, tag="ones_row")
            nc.vector.memset(ones_row[:], 1.0)
            gb_row = cst.tile([1, K], F32, tag="gb_row")
            nc.sync.dma_start(gb_row[:], gate_b[None, :])
            gb_ps = trp.tile([P, D], F32, tag="tr")
            nc.tensor.matmul(gb_ps[:, :K], ones_row[:], gb_row[:])
            gb_bc = cst.tile([P, K], F32, tag="gb_bc")
            nc.scalar.activation(gb_bc[:], gb_ps[:, :K], AF.Copy)

        bl_sb = [None] * 4
        if any_bias:
            for li in range(4):
                if has_bias[li]:
                    blt = cst.tile([K, D], FR, tag=f"bl{li}")
                    nc.sync.dma_start(blt[:], bs[li][:, :])
                    bl_sb[li] = blt
            alphaT = cst.tile([K, R], FR, tag="alphaT")

        alpha = cst.tile([P, NT * K], F32, tag="alpha")

        # ---- prologue: load x tiles, transpose to feature-major, gate ----
        hT = {}
        for t in range(NT):
            xb = xbp.tile([P, D], F32, tag="xb")
            nc.sync.dma_start(xb[:], x[t * P:(t + 1) * P, :])
            trt = trp.tile([P, NCH, P], F32, tag="tr")
            for c in range(NCH):
                nc.tensor.transpose(trt[:, c, :], xb[:, c * P:(c + 1) * P], ident[:])
            ht = htp.tile([P, NCH, P], SD, tag="ht")
            nc.scalar.activation(ht[:], trt[:], AF.Copy)
            hT[(0, t)] = ht

            # gate for this tile (batch-major logits)
            lg = zp.tile([P, D], F32, tag="z")
            for c in range(NCH):
                nc.tensor.matmul(
                    lg[:, :K],
                    ht[:, c, :],
                    gw[:, c * K:(c + 1) * K],
                    start=(c == 0),
                    stop=(c == NCH - 1),
                )
            ex = smp.tile([P, K], F32, tag="ex")
            if has_gate_b:
                nc.vector.scalar_tensor_tensor(
                    ex[:], lg[:, :K], 1.0, gb_bc[:], op0=ALU.mult, op1=ALU.add
                )
                nc.scalar.activation(ex[:], ex[:], AF.Exp)
            else:
                nc.scalar.activation(ex[:], lg[:, :K], AF.Exp)
            ssum = smp.tile([P, 1], F32, tag="ssum")
            nc.vector.reduce_sum(ssum[:], ex[:], axis=mybir.AxisListType.X)
            rec = smp.tile([P, 1], F32, tag="rec")
            nc.vector.reciprocal(rec[:], ssum[:])
            nc.vector.tensor_scalar_mul(alpha[:, t * K:(t + 1) * K], ex[:], rec[:])

            if any_bias:
                # alphaT[:, t*128:(t+1)*128] = alpha_tile.T (8 x 128)
                at_ps = trp.tile([P, NCH, P], F32, tag="tr")
                nc.tensor.transpose(
                    at_ps[:K, 0, :], alpha[:, t * K:(t + 1) * K], ident[:]
                )
                nc.scalar.activation(
                    alphaT[:, t * P:(t + 1) * P], at_ps[:K, 0, :], AF.Copy
                )

        # ---- 4 MoIE blocks (x repeat for timing builds) ----
        # tile t's transposes are deferred until after tile t+1's GEMMs
        # (across layer boundaries too): the PE executes in order, so
        # transposing right after tile t's GEMMs would stall the PE on
        # the combine chain.
        pend = []

        def flush_pending():
            accb, dst, af = pend.pop(0)
            trt = trp.tile([P, NCH, P], SD, tag="tr")
            for c in range(NCH):
                nc.tensor.transpose(
                    trt[:, c, :], accb[:, c * P:(c + 1) * P], identb[:]
                )
            ht_n = htp.tile([P, NCH, P], SD, tag="ht")
            nc.scalar.activation(ht_n[:], trt[:], af)
            hT[dst] = ht_n

        for gli in range(4 * repeat):
            li = gli % 4
            last = gli == 4 * repeat - 1
            # stream this layer's weights (reused across all 16 batch tiles);
            # one big DMA per expert: [128, 4 chunks, 512]
            wsr = ws[li].rearrange("k (c p) o -> k p c o", p=P)
            wt = {}
            for k in range(K):
                w_t = wpool.tile([P, NCH, D], MD, tag="w", name=f"w_{gli}_{k}")
                nc.sync.dma_start(w_t[:], wsr[k])
                wt[k] = w_t

            for t in range(NT):
                bias_sb = None
                if has_bias[li]:
                    b_ps = trp.tile([P, D], F32, tag="tr")
                    nc.tensor.matmul(
                        b_ps[:],
                        alphaT[:, t * P:(t + 1) * P],
                        bl_sb[li][:],
                    )
                    bias_sb = smp.tile([P, D], F32, tag="bias_sb")
                    nc.scalar.activation(bias_sb[:], b_ps[:], AF.Copy)

                while (gli, t) not in hT:
                    flush_pending()
                ht_in = hT[(gli, t)]
                zs = []
                for k in range(K):
                    z = zp.tile([P, D], F32, tag="z", name=f"z_{gli}_{t}_{k}")
                    for c in range(NCH):
                        nc.tensor.matmul(
                            z[:],
                            ht_in[:, c, :],
                            wt[k][:, c, :],
                            start=(c == 0),
                            stop=(c == NCH - 1),
                        )
                    zs.append(z)

                if mode in ("pe_only",):
                    continue

                # weighted sum over experts: VectorE STT chain for the head
                # experts; tail experts scaled on ScalarE with partial sums
                # on GpSimd. Final add writes bf16 (except last layer, which
                # is DMAed out in fp32).
                n_dve = K - gp_offload
                gsum = None
                for j, k in enumerate(range(n_dve, K)):
                    a_ap = alpha[:, t * K + k:t * K + k + 1]
                    sk = smp.tile([P, D], F32, tag="sk", bufs=3)
                    nc.scalar.activation(sk[:], zs[k][:], AF.Copy, scale=a_ap)
                    if j == 0:
                        gsum = sk
                    else:
                        gs2 = smp.tile([P, D], F32, tag="gs", bufs=2)
                        nc.gpsimd.tensor_add(gs2[:], gsum[:], sk[:])
                        gsum = gs2
                # two parallel STT chains on DVE so z banks free sooner
                acc = accp.tile([P, D], F32, tag="acc")
                acc2 = None
                for k in range(n_dve):
                    a_ap = alpha[:, t * K + k:t * K + k + 1]
                    if split and k == 1:
                        acc2 = accp.tile([P, D], F32, tag="acc2", bufs=3)
                        nc.vector.tensor_scalar_mul(acc2[:], zs[k][:], a_ap)
                        continue
                    tgt = acc2 if (split and acc2 is not None and k % 2 == 1) else acc
                    if k == 0:
                        if bias_sb is not None:
                            nc.vector.scalar_tensor_tensor(
                                acc[:], zs[0][:], a_ap, bias_sb[:],
                                op0=ALU.mult, op1=ALU.add,
                            )
                        else:
                            nc.vector.tensor_scalar_mul(acc[:], zs[0][:], a_ap)
                    else:
                        nc.vector.scalar_tensor_tensor(
                            tgt[:], zs[k][:], a_ap, tgt[:],
                            op0=ALU.mult, op1=ALU.add,
                        )
                if acc2 is not None:
                    # fold the second chain into the GpSimd partial sum
                    gs3 = smp.tile([P, D], F32, tag="gs3", bufs=2)
                    nc.gpsimd.tensor_add(gs3[:], gsum[:], acc2[:])
                    gsum = gs3

                if not last:
                    # final combine add writes bf16 directly (no extra
                    # rounding vs the bf16 matmul input); relu fuses into
                    # the PSUM->SBUF eviction copy after the transposes
                    accb = accbp.tile([P, D], SD, tag="accb")
                    if gsum is not None:
                        nc.vector.tensor_add(accb[:], acc[:], gsum[:])
                    else:
                        nc.vector.tensor_copy(accb[:], acc[:])
                    pend.append((accb, (gli + 1, t),
                                 AF.Relu if li < 3 else AF.Copy))
                    if len(pend) > 1:
                        flush_pending()
                else:
                    if gsum is not None:
                        nc.vector.tensor_add(acc[:], acc[:], gsum[:])
                    nc.sync.dma_start(out[t * P:(t + 1) * P, :], acc[:])

        while pend:
            flush_pending()

    nc.compile()
    return nc


_CACHE = {}


def _get_nc(key):
    if key not in _CACHE:
        _CACHE[key] = _build(key[0], key[1])
    return _CACHE[key]


def _to_md(a, bf16=False):
    a = np.ascontiguousarray(np.asarray(a, dtype=np.float32))
    if not bf16:
        return a
    import ml_dtypes

    return np.ascontiguousarray(a.astype(ml_dtypes.bfloat16))


def kernel(**inputs):
    x = np.ascontiguousarray(np.asarray(inputs["x"], dtype=np.float32))
    gate_w = _to_md(inputs["gate_w"])
    gate_b = np.ascontiguousarray(np.asarray(inputs["gate_b"], dtype=np.float32))
    wlist = [_to_md(inputs[n]) for n in W_NAMES]
    blist = [np.ascontiguousarray(np.asarray(inputs[n], dtype=np.float32)) for n in B_NAMES]

    has_gate_b = bool(np.any(gate_b))
    has_bias = tuple(bool(np.any(b)) for b in blist)
    nc = _get_nc((has_gate_b, has_bias))

    shared = {"gate_w": gate_w, "gate_b": gate_b}
    for n, w in zip(W_NAMES, wlist):
        shared[n] = w
    for n, b in zip(B_NAMES, blist):
        shared[n] = b

    core_ids = list(range(NCORES))
    in_maps = [dict(shared, x=x[i * R:(i + 1) * R]) for i in core_ids]
    res = run_bass_kernel_spmd(nc, in_maps, core_ids)
    return np.concatenate([res.results[i]["out"] for i in core_ids], axis=0)


if __name__ == "__main__":
    rng = np.random.default_rng(0)
    ins = {
        "x": rng.standard_normal((B, D), dtype=np.float32),
        "gate_w": rng.standard_normal((D, K), dtype=np.float32) * 0.02,
        "gate_b": np.zeros((K,), np.float32),
    }
    for n in W_NAMES:
        ins[n] = rng.standard_normal((K, D, D), dtype=np.float32) * 0.02
    for n in B_NAMES:
        ins[n] = np.zeros((K, D), np.float32)
    y = kernel(**ins)
    print("out", y.shape, y.dtype, float(np.abs(y).max()))


# revision 13
# speedup vs baseline: 1.0694x; 1.0694x over previous
"""Trainium2 Bass kernel for the BMoIE (dense mixture-of-experts) network.

Network (per sample):
    alpha = softmax(x @ gate_w + gate_b)                       # [B, 8]
    h = relu(sum_k alpha_k * (h @ w_l[k] + b_l[k]))            # 3 hidden blocks
    out = sum_k alpha_k * (h @ wo[k] + bo[k])                  # output block

Strategy: data-parallel over 8 NeuronCores (2048 rows each, SPMD, no
collectives). On each core, activations are kept feature-major ("hT",
[128 feat-partitions x batch]) so they can be the stationary matmul
operand; expert weights stream in natural layout as the moving operand,
producing batch-major per-expert outputs in PSUM. Matmul operands are
bf16 (full PE speed, half the HBM traffic of fp32; rel-l2 ~5e-3 vs the
2e-2 budget). The alpha-weighted sum over the 8 experts is split across
engines: 5 experts as two parallel VectorE scalar_tensor_tensor chains
(fp32 accumulate), 3 experts scaled on ScalarE with partial sums on
GpSimd. The combined result is written as bf16 (the next layer's matmul
input is bf16 anyway), so the PE re-transposes to feature-major run at
1 cycle/row; each tile's transposes are deferred until after the next
tile's GEMMs so the in-order PE never stalls on the combine chain.
Weights load as one DMA per (layer, expert) for few, large transfers.
"""

import sys

sys.path.insert(0, "/opt/trn_rl_repo")

import numpy as np

import concourse.bass as bass
import concourse.mybir as mybir
import concourse.tile as tile
from concourse import bacc
from concourse.bass_utils import run_bass_kernel_spmd
from concourse.masks import make_identity

P = 128           # partitions
D = 512           # model dim (= hidden dim)
K = 8             # experts
NCORES = 8
B = 16384
R = B // NCORES   # rows per core
NT = R // P       # 16 batch tiles per core
NCH = D // P      # 4 feature chunks
F32 = mybir.dt.float32
FR = mybir.dt.float32r
BF16 = mybir.dt.bfloat16
AF = mybir.ActivationFunctionType
ALU = mybir.AluOpType

W_NAMES = ("w0", "w1", "w2", "wo")
B_NAMES = ("b0", "b1", "b2", "bo")


def _build(has_gate_b, has_bias, w_bufs=None, ht_bufs=None, z_bufs=7, repeat=1,
           mode="full", gp_offload=3, bf16=True, split=True):
    """Trace + compile the per-core kernel. has_bias is a 4-tuple of bools.

    repeat>1 runs the whole 4-layer stack that many times (same weights,
    full DMA traffic each time) — used only for timing measurements.
    mode: "full" | "pe_only" (no combine/evict) — ablation variants for
    timing only (wrong results except "full").
    gp_offload: number of experts whose scale runs on ScalarE with the
    partial sum on GpSimd (reduces VectorE op count).
    """
    from contextlib import ExitStack

    MD = BF16 if bf16 else FR  # matmul operand dtype
    SD = MD                    # stationary (activations) dtype
    if w_bufs is None:
        w_bufs = 20 if bf16 else 12
    if ht_bufs is None:
        ht_bufs = 24 if bf16 else 20

    nc = bacc.Bacc("TRN2", target_bir_lowering=False, num_devices=NCORES)
    x = nc.dram_tensor("x", [R, D], F32, kind="ExternalInput")
    gate_w = nc.dram_tensor("gate_w", [D, K], MD, kind="ExternalInput")
    gate_b = nc.dram_tensor("gate_b", [K], F32, kind="ExternalInput")
    ws = [nc.dram_tensor(n, [K, D, D], MD, kind="ExternalInput") for n in W_NAMES]
    bs = [nc.dram_tensor(n, [K, D], FR, kind="ExternalInput") for n in B_NAMES]
    out = nc.dram_tensor("out", [R, D], F32, kind="ExternalOutput")

    any_bias = any(has_bias)

    with tile.TileContext(nc) as tc, ExitStack() as ctx:
        cst = ctx.enter_context(tc.tile_pool(name="cst", bufs=1))
        wpool = ctx.enter_context(tc.tile_pool(name="wpool", bufs=w_bufs))
        htp = ctx.enter_context(tc.tile_pool(name="htp", bufs=ht_bufs))
        accp = ctx.enter_context(tc.tile_pool(name="accp", bufs=3))
        accbp = ctx.enter_context(tc.tile_pool(name="accbp", bufs=3))
        xbp = ctx.enter_context(tc.tile_pool(name="xbp", bufs=3))
        smp = ctx.enter_context(tc.tile_pool(name="smp", bufs=4))
        zp = ctx.enter_context(tc.tile_pool(name="zp", bufs=z_bufs, space="PSUM"))
        trp = ctx.enter_context(tc.tile_pool(name="trp", bufs=8 - z_bufs, space="PSUM"))

        ident = cst.tile([P, P], F32, tag="ident")
        make_identity(nc, ident[:])
        identb = cst.tile([P, P], SD, tag="identb")
        nc.scalar.activation(identb[:], ident[:], AF.Copy)

        # gate_w [512, 8] -> [128, 4*8] (chunk c at cols c*8..)
        gw = cst.tile([P, NCH * K], MD, tag="gw")
        for c in range(NCH):
            nc.sync.dma_start(gw[:, c * K:(c + 1) * K], gate_w[c * P:(c + 1) * P, :])

        gb_bc = None
        if has_gate_b:
            ones_row = cst.tile([1, P], F32, tag="ones_row")
            nc.vector.memset(ones_row[:], 1.0)
            gb_row = cst.tile([1, K], F32, tag="gb_row")
            nc.sync.dma_start(gb_row[:], gate_b[None, :])
            gb_ps = trp.tile([P, D], F32, tag="tr")
            nc.tensor.matmul(gb_ps[:, :K], ones_row[:], gb_row[:])
            gb_bc = cst.tile([P, K], F32, tag="gb_bc")
            nc.scalar.activation(gb_bc[:], gb_ps[:, :K], AF.Copy)

        bl_sb = [None] * 4
        if any_bias:
            for li in range(4):
                if has_bias[li]:
                    blt = cst.tile([K, D], FR, tag=f"bl{li}")
                    nc.sync.dma_start(blt[:], bs[li][:, :])
                    bl_sb[li] = blt
            alphaT = cst.tile([K, R], FR, tag="alphaT")

        alpha = cst.tile([P, NT * K], F32, tag="alpha")

        # ---- prologue: load x tiles, transpose to feature-major, gate ----
        hT = {}
        for t in range(NT):
            xb = xbp.tile([P, D], F32, tag="xb")
            nc.sync.dma_start(xb[:], x[t * P:(t + 1) * P, :])
            trt = trp.tile([P, NCH, P], F32, tag="tr")
            for c in range(NCH):
                nc.tensor.transpose(trt[:, c, :], xb[:, c * P:(c + 1) * P], ident[:])
            ht = htp.tile([P, NCH, P], SD, tag="ht")
            nc.scalar.activation(ht[:], trt[:], AF.Copy)
            hT[(0, t)] = ht

            # gate for this tile (batch-major logits)
            lg = zp.tile([P, D], F32, tag="z")
            for c in range(NCH):
                nc.tensor.matmul(
                    lg[:, :K],
                    ht[:, c, :],
                    gw[:, c * K:(c + 1) * K],
                    start=(c == 0),
                    stop=(c == NCH - 1),
                )
            ex = smp.tile([P, K], F32, tag="ex")
            if has_gate_b:
                nc.vector.scalar_tensor_tensor(
                    ex[:], lg[:, :K], 1.0, gb_bc[:], op0=ALU.mult, op1=ALU.add
                )
                nc.scalar.activation(ex[:], ex[:], AF.Exp)
            else:
                nc.scalar.activation(ex[:], lg[:, :K], AF.Exp)
            ssum = smp.tile([P, 1], F32, tag="ssum")
            nc.vector.reduce_sum(ssum[:], ex[:], axis=mybir.AxisListType.X)
            rec = smp.tile([P, 1], F32, tag="rec")
            nc.vector.reciprocal(rec[:], ssum[:])
            nc.vector.tensor_scalar_mul(alpha[:, t * K:(t + 1) * K], ex[:], rec[:])

            if any_bias:
                # alphaT[:, t*128:(t+1)*128] = alpha_tile.T (8 x 128)
                at_ps = trp.tile([P, NCH, P], F32, tag="tr")
                nc.tensor.transpose(
                    at_ps[:K, 0, :], alpha[:, t * K:(t + 1) * K], ident[:]
                )
                nc.scalar.activation(
                    alphaT[:, t * P:(t + 1) * P], at_ps[:K, 0, :], AF.Copy
                )

        # ---- 4 MoIE blocks (x repeat for timing builds) ----
        # tile t's transposes are deferred until after tile t+1's GEMMs
        # (across layer boundaries too): the PE executes in order, so
        # transposing right after tile t's GEMMs would stall the PE on
        # the combine chain.
        pend = []

        def flush_pending():
            accb, dst, af = pend.pop(0)
            trt = trp.tile([P, NCH, P], SD, tag="tr")
            for c in range(NCH):
                nc.tensor.transpose(
                    trt[:, c, :], accb[:, c * P:(c + 1) * P], identb[:]
                )
            ht_n = htp.tile([P, NCH, P], SD, tag="ht")
            nc.scalar.activation(ht_n[:], trt[:], af)
            hT[dst] = ht_n

        for gli in range(4 * repeat):
            li = gli % 4
            last = gli == 4 * repeat - 1
            # stream this layer's weights (reused across all 16 batch tiles);
            # one big DMA per expert: [128, 4 chunks, 512]
            wsr = ws[li].rearrange("k (c p) o -> k p c o", p=P)
            wt = {}
            for k in range(K):
                w_t = wpool.tile([P, NCH, D], MD, tag="w", name=f"w_{gli}_{k}")
                nc.sync.dma_start(w_t[:], wsr[k])
                wt[k] = w_t

            for t in range(NT):
                bias_sb = None
                if has_bias[li]:
                    b_ps = trp.tile([P, D], F32, tag="tr")
                    nc.tensor.matmul(
                        b_ps[:],
                        alphaT[:, t * P:(t + 1) * P],
                        bl_sb[li][:],
                    )
                    bias_sb = smp.tile([P, D], F32, tag="bias_sb")
                    nc.scalar.activation(bias_sb[:], b_ps[:], AF.Copy)

                while (gli, t) not in hT:
                    flush_pending()
                ht_in = hT[(gli, t)]
                zs = []
                for k in range(K):
                    z = zp.tile([P, D], F32, tag="z", name=f"z_{gli}_{t}_{k}")
                    for c in range(NCH):
                        nc.tensor.matmul(
                            z[:],
                            ht_in[:, c, :],
                            wt[k][:, c, :],
                            start=(c == 0),
                            stop=(c == NCH - 1),
                        )
                    zs.append(z)

                if mode in ("pe_only",):
                    continue

                # weighted sum over experts: VectorE STT chain for the head
                # experts; tail experts scaled on ScalarE with partial sums
                # on GpSimd. Final add writes bf16 (except last layer, which
                # is DMAed out in fp32).
                n_dve = K - gp_offload
                gsum = None
                for j, k in enumerate(range(n_dve, K)):
                    a_ap = alpha[:, t * K + k:t * K + k + 1]
                    sk = smp.tile([P, D], F32, tag="sk", bufs=3)
                    nc.scalar.activation(sk[:], zs[k][:], AF.Copy, scale=a_ap)
                    if j == 0:
                        gsum = sk
                    else:
                        gs2 = smp.tile([P, D], F32, tag="gs", bufs=2)
                        nc.gpsimd.tensor_add(gs2[:], gsum[:], sk[:])
                        gsum = gs2
                # two parallel STT chains on DVE so z banks free sooner
                acc = accp.tile([P, D], F32, tag="acc")
                acc2 = None
                for k in range(n_dve):
                    a_ap = alpha[:, t * K + k:t * K + k + 1]
                    if split and k == 1:
                        acc2 = accp.tile([P, D], F32, tag="acc2", bufs=3)
                        nc.vector.tensor_scalar_mul(acc2[:], zs[k][:], a_ap)
                        continue
                    tgt = acc2 if (split and acc2 is not None and k % 2 == 1) else acc
                    if k == 0:
                        if bias_sb is not None:
                            nc.vector.scalar_tensor_tensor(
                                acc[:], zs[0][:], a_ap, bias_sb[:],
                                op0=ALU.mult, op1=ALU.add,
                            )
                        else:
                            nc.vector.tensor_scalar_mul(acc[:], zs[0][:], a_ap)
                    else:
                        nc.vector.scalar_tensor_tensor(
                            tgt[:], zs[k][:], a_ap, tgt[:],
                            op0=ALU.mult, op1=ALU.add,
                        )
                if acc2 is not None:
                    # fold the second chain into the GpSimd partial sum
                    gs3 = smp.tile([P, D], F32, tag="gs3", bufs=2)
                    nc.gpsimd.tensor_add(gs3[:], gsum[:], acc2[:])
                    gsum = gs3

                if not last:
                    # final combine add writes bf16 directly (no extra
                    # rounding vs the bf16 matmul input); relu fuses into
                    # the PSUM->SBUF eviction copy after the transposes
                    accb = accbp.tile([P, D], SD, tag="accb")
                    if gsum is not None:
                        nc.vector.tensor_add(accb[:], acc[:], gsum[:])
                    else:
                        nc.vector.tensor_copy(accb[:], acc[:])
                    pend.append((accb, (gli + 1, t),
                                 AF.Relu if li < 3 else AF.Copy))
                    if len(pend) > 1:
                        flush_pending()
                else:
                    if gsum is not None:
                        nc.vector.tensor_add(acc[:], acc[:], gsum[:])
                    nc.sync.dma_start(out[t * P:(t + 1) * P, :], acc[:])

        while pend:
            flush_pending()

    nc.compile()
    return nc


_CACHE = {}


def _get_nc(key):
    if key not in _CACHE:
        _CACHE[key] = _build(key[0], key[1])
    return _CACHE[key]


def _to_md(a):
    import ml_dtypes

    return np.ascontiguousarray(np.asarray(a, dtype=np.float32).astype(
        ml_dtypes.bfloat16))


def kernel(**inputs):
    x = np.ascontiguousarray(np.asarray(inputs["x"], dtype=np.float32))
    gate_w = _to_md(inputs["gate_w"])
    gate_b = np.ascontiguousarray(np.asarray(inputs["gate_b"], dtype=np.float32))
    wlist = [_to_md(inputs[n]) for n in W_NAMES]
    blist = [np.ascontiguousarray(np.asarray(inputs[n], dtype=np.float32)) for n in B_NAMES]

    has_gate_b = bool(np.any(gate_b))
    has_bias = tuple(bool(np.any(b)) for b in blist)
    nc = _get_nc((has_gate_b, has_bias))

    shared = {"gate_w": gate_w, "gate_b": gate_b}
    for n, w in zip(W_NAMES, wlist):
        shared[n] = w
    for n, b in zip(B_NAMES, blist):
        shared[n] = b

    core_ids = list(range(NCORES))
    in_maps = [dict(shared, x=x[i * R:(i + 1) * R]) for i in core_ids]
    res = run_bass_kernel_spmd(nc, in_maps, core_ids)
    return np.concatenate([res.results[i]["out"] for i in core_ids], axis=0)


if __name__ == "__main__":
    rng = np.random.default_rng(0)
    ins = {
        "x": rng.standard_normal((B, D), dtype=np.float32),
        "gate_w": rng.standard_normal((D, K), dtype=np.float32) * 0.02,
        "gate_b": np.zeros((K,), np.float32),
    }
    for n in W_NAMES:
        ins[n] = rng.standard_normal((K, D, D), dtype=np.float32) * 0.02
    for n in B_NAMES:
        ins[n] = np.zeros((K, D), np.float32)
    y = kernel(**ins)
    print("out", y.shape, y.dtype, float(np.abs(y).max()))


# revision 15
# speedup vs baseline: 1.0867x; 1.0162x over previous
"""Trainium2 Bass kernel for the BMoIE (dense mixture-of-experts) network.

Network (per sample):
    alpha = softmax(x @ gate_w + gate_b)                       # [B, 8]
    h = relu(sum_k alpha_k * (h @ w_l[k] + b_l[k]))            # 3 hidden blocks
    out = sum_k alpha_k * (h @ wo[k] + bo[k])                  # output block

Strategy: data-parallel over 8 NeuronCores (2048 rows each, SPMD, no
collectives). On each core, activations are kept feature-major ("hT",
[128 feat-partitions x batch]) so they can be the stationary matmul
operand; expert weights stream in natural layout as the moving operand,
producing batch-major per-expert outputs in PSUM. Matmul operands are
bf16 (full PE speed, half the HBM traffic of fp32; rel-l2 ~5e-3 vs the
2e-2 budget). The alpha-weighted sum over the 8 experts is split across
engines: experts 0,2,4,6,7 as a VectorE scalar_tensor_tensor chain
(fp32 accumulate) ending in a one-op tail, while experts 1,3,5 are
scaled on ScalarE and partial-summed on GpSimd mid-tile (GpSimd launch
latency never sits on the critical tail). The combined result is
written as bf16 (the next layer's matmul input is bf16 anyway), so the
PE re-transposes to feature-major run at 1 cycle/row; each tile's
transposes are deferred two tiles behind the GEMMs so the in-order PE
never stalls on the combine chain. Weights load as one DMA per
(layer, expert) for few, large transfers.
"""

import sys

sys.path.insert(0, "/opt/trn_rl_repo")

import numpy as np

import concourse.bass as bass
import concourse.mybir as mybir
import concourse.tile as tile
from concourse import bacc
from concourse.bass_utils import run_bass_kernel_spmd
from concourse.masks import make_identity

P = 128           # partitions
D = 512           # model dim (= hidden dim)
K = 8             # experts
NCORES = 8
B = 16384
R = B // NCORES   # rows per core
NT = R // P       # 16 batch tiles per core
NCH = D // P      # 4 feature chunks
F32 = mybir.dt.float32
FR = mybir.dt.float32r
BF16 = mybir.dt.bfloat16
AF = mybir.ActivationFunctionType
ALU = mybir.AluOpType

W_NAMES = ("w0", "w1", "w2", "wo")
B_NAMES = ("b0", "b1", "b2", "bo")


def _build(has_gate_b, has_bias, w_bufs=None, ht_bufs=None, z_bufs=7, repeat=1,
           mode="full", gp_offload=3, bf16=True, split=True):
    """Trace + compile the per-core kernel. has_bias is a 4-tuple of bools.

    repeat>1 runs the whole 4-layer stack that many times (same weights,
    full DMA traffic each time) — used only for timing measurements.
    mode: "full" | "pe_only" (no combine/evict) — ablation variants for
    timing only (wrong results except "full").
    gp_offload: number of experts whose scale runs on ScalarE with the
    partial sum on GpSimd (reduces VectorE op count).
    """
    from contextlib import ExitStack

    MD = BF16 if bf16 else FR  # matmul operand dtype
    SD = MD                    # stationary (activations) dtype
    if w_bufs is None:
        w_bufs = 20 if bf16 else 12
    if ht_bufs is None:
        ht_bufs = 24 if bf16 else 20

    nc = bacc.Bacc("TRN2", target_bir_lowering=False, num_devices=NCORES)
    x = nc.dram_tensor("x", [R, D], F32, kind="ExternalInput")
    gate_w = nc.dram_tensor("gate_w", [D, K], MD, kind="ExternalInput")
    gate_b = nc.dram_tensor("gate_b", [K], F32, kind="ExternalInput")
    ws = [nc.dram_tensor(n, [K, D, D], MD, kind="ExternalInput") for n in W_NAMES]
    bs = [nc.dram_tensor(n, [K, D], FR, kind="ExternalInput") for n in B_NAMES]
    out = nc.dram_tensor("out", [R, D], F32, kind="ExternalOutput")

    any_bias = any(has_bias)

    with tile.TileContext(nc) as tc, ExitStack() as ctx:
        cst = ctx.enter_context(tc.tile_pool(name="cst", bufs=1))
        wpool = ctx.enter_context(tc.tile_pool(name="wpool", bufs=w_bufs))
        htp = ctx.enter_context(tc.tile_pool(name="htp", bufs=ht_bufs))
        accp = ctx.enter_context(tc.tile_pool(name="accp", bufs=3))
        accbp = ctx.enter_context(tc.tile_pool(name="accbp", bufs=4))
        xbp = ctx.enter_context(tc.tile_pool(name="xbp", bufs=3))
        smp = ctx.enter_context(tc.tile_pool(name="smp", bufs=4))
        zp = ctx.enter_context(tc.tile_pool(name="zp", bufs=z_bufs, space="PSUM"))
        trp = ctx.enter_context(tc.tile_pool(name="trp", bufs=8 - z_bufs, space="PSUM"))

        ident = cst.tile([P, P], F32, tag="ident")
        make_identity(nc, ident[:])
        identb = cst.tile([P, P], SD, tag="identb")
        nc.scalar.activation(identb[:], ident[:], AF.Copy)

        # gate_w [512, 8] -> [128, 4*8] (chunk c at cols c*8..)
        gw = cst.tile([P, NCH * K], MD, tag="gw")
        for c in range(NCH):
            nc.sync.dma_start(gw[:, c * K:(c + 1) * K], gate_w[c * P:(c + 1) * P, :])

        gb_bc = None
        if has_gate_b:
            ones_row = cst.tile([1, P], F32, tag="ones_row")
            nc.vector.memset(ones_row[:], 1.0)
            gb_row = cst.tile([1, K], F32, tag="gb_row")
            nc.sync.dma_start(gb_row[:], gate_b[None, :])
            gb_ps = trp.tile([P, D], F32, tag="tr")
            nc.tensor.matmul(gb_ps[:, :K], ones_row[:], gb_row[:])
            gb_bc = cst.tile([P, K], F32, tag="gb_bc")
            nc.scalar.activation(gb_bc[:], gb_ps[:, :K], AF.Copy)

        bl_sb = [None] * 4
        if any_bias:
            for li in range(4):
                if has_bias[li]:
                    blt = cst.tile([K, D], FR, tag=f"bl{li}")
                    nc.sync.dma_start(blt[:], bs[li][:, :])
                    bl_sb[li] = blt
            alphaT = cst.tile([K, R], FR, tag="alphaT")

        alpha = cst.tile([P, NT * K], F32, tag="alpha")

        # ---- prologue: load x tiles, transpose to feature-major, gate ----
        hT = {}
        for t in range(NT):
            xb = xbp.tile([P, D], F32, tag="xb")
            nc.sync.dma_start(xb[:], x[t * P:(t + 1) * P, :])
            trt = trp.tile([P, NCH, P], F32, tag="tr")
            for c in range(NCH):
                nc.tensor.transpose(trt[:, c, :], xb[:, c * P:(c + 1) * P], ident[:])
            ht = htp.tile([P, NCH, P], SD, tag="ht")
            nc.scalar.activation(ht[:], trt[:], AF.Copy)
            hT[(0, t)] = ht

            # gate for this tile (batch-major logits)
            lg = zp.tile([P, D], F32, tag="z")
            for c in range(NCH):
                nc.tensor.matmul(
                    lg[:, :K],
                    ht[:, c, :],
                    gw[:, c * K:(c + 1) * K],
                    start=(c == 0),
                    stop=(c == NCH - 1),
                )
            ex = smp.tile([P, K], F32, tag="ex")
            if has_gate_b:
                nc.vector.scalar_tensor_tensor(
                    ex[:], lg[:, :K], 1.0, gb_bc[:], op0=ALU.mult, op1=ALU.add
                )
                nc.scalar.activation(ex[:], ex[:], AF.Exp)
            else:
                nc.scalar.activation(ex[:], lg[:, :K], AF.Exp)
            ssum = smp.tile([P, 1], F32, tag="ssum")
            nc.vector.reduce_sum(ssum[:], ex[:], axis=mybir.AxisListType.X)
            rec = smp.tile([P, 1], F32, tag="rec")
            nc.vector.reciprocal(rec[:], ssum[:])
            nc.vector.tensor_scalar_mul(alpha[:, t * K:(t + 1) * K], ex[:], rec[:])

            if any_bias:
                # alphaT[:, t*128:(t+1)*128] = alpha_tile.T (8 x 128)
                at_ps = trp.tile([P, NCH, P], F32, tag="tr")
                nc.tensor.transpose(
                    at_ps[:K, 0, :], alpha[:, t * K:(t + 1) * K], ident[:]
                )
                nc.scalar.activation(
                    alphaT[:, t * P:(t + 1) * P], at_ps[:K, 0, :], AF.Copy
                )

        # ---- 4 MoIE blocks (x repeat for timing builds) ----
        # tile t's transposes are deferred until after tile t+1's GEMMs
        # (across layer boundaries too): the PE executes in order, so
        # transposing right after tile t's GEMMs would stall the PE on
        # the combine chain.
        pend = []

        def flush_pending():
            accb, dst, af = pend.pop(0)
            trt = trp.tile([P, NCH, P], SD, tag="tr")
            for c in range(NCH):
                nc.tensor.transpose(
                    trt[:, c, :], accb[:, c * P:(c + 1) * P], identb[:]
                )
            ht_n = htp.tile([P, NCH, P], SD, tag="ht")
            nc.scalar.activation(ht_n[:], trt[:], af)
            hT[dst] = ht_n

        for gli in range(4 * repeat):
            li = gli % 4
            last = gli == 4 * repeat - 1
            # stream this layer's weights (reused across all 16 batch tiles);
            # one big DMA per expert: [128, 4 chunks, 512]
            wsr = ws[li].rearrange("k (c p) o -> k p c o", p=P)
            wt = {}
            for k in range(K):
                w_t = wpool.tile([P, NCH, D], MD, tag="w", name=f"w_{gli}_{k}")
                nc.sync.dma_start(w_t[:], wsr[k])
                wt[k] = w_t

            for t in range(NT):
                bias_sb = None
                if has_bias[li]:
                    b_ps = trp.tile([P, D], F32, tag="tr")
                    nc.tensor.matmul(
                        b_ps[:],
                        alphaT[:, t * P:(t + 1) * P],
                        bl_sb[li][:],
                    )
                    bias_sb = smp.tile([P, D], F32, tag="bias_sb")
                    nc.scalar.activation(bias_sb[:], b_ps[:], AF.Copy)

                while (gli, t) not in hT:
                    flush_pending()
                ht_in = hT[(gli, t)]
                zs = []
                for k in range(K):
                    z = zp.tile([P, D], F32, tag="z", name=f"z_{gli}_{t}_{k}")
                    for c in range(NCH):
                        nc.tensor.matmul(
                            z[:],
                            ht_in[:, c, :],
                            wt[k][:, c, :],
                            start=(c == 0),
                            stop=(c == NCH - 1),
                        )
                    zs.append(z)

                if mode in ("pe_only",):
                    continue

                # weighted sum over experts: VectorE STT chain for the head
                # experts; tail experts scaled on ScalarE with partial sums
                # on GpSimd. Final add writes bf16 (except last layer, which
                # is DMAed out in fp32).
                # side chain on ScalarE+GpSimd takes EARLY experts (1,3,5)
                # so the GpSimd adds land mid-tile; the DVE STT chain takes
                # 0,2,4,6,7 and ends with a single add tail after z7 — the
                # deferred transposes then never wait on slow engines.
                side = (1, 3, 5)
                sk = {}
                for k in side:
                    a_ap = alpha[:, t * K + k:t * K + k + 1]
                    skt = smp.tile([P, D], F32, tag="sk", bufs=4)
                    nc.scalar.activation(skt[:], zs[k][:], AF.Copy, scale=a_ap)
                    sk[k] = skt
                gs2 = smp.tile([P, D], F32, tag="gs", bufs=2)
                nc.gpsimd.tensor_add(gs2[:], sk[1][:], sk[3][:])
                gsum = smp.tile([P, D], F32, tag="gs3", bufs=2)
                nc.gpsimd.tensor_add(gsum[:], gs2[:], sk[5][:])
                acc = accp.tile([P, D], F32, tag="acc")
                first = True
                for k in range(K):
                    if k in side:
                        continue
                    a_ap = alpha[:, t * K + k:t * K + k + 1]
                    if first:
                        first = False
                        if bias_sb is not None:
                            nc.vector.scalar_tensor_tensor(
                                acc[:], zs[k][:], a_ap, bias_sb[:],
                                op0=ALU.mult, op1=ALU.add,
                            )
                        else:
                            nc.vector.tensor_scalar_mul(acc[:], zs[k][:], a_ap)
                    else:
                        nc.vector.scalar_tensor_tensor(
                            acc[:], zs[k][:], a_ap, acc[:],
                            op0=ALU.mult, op1=ALU.add,
                        )

                if not last:
                    # final combine add writes bf16 directly (no extra
                    # rounding vs the bf16 matmul input); relu fuses into
                    # the PSUM->SBUF eviction copy after the transposes
                    accb = accbp.tile([P, D], SD, tag="accb")
                    if gsum is not None:
                        nc.vector.tensor_add(accb[:], acc[:], gsum[:])
                    else:
                        nc.vector.tensor_copy(accb[:], acc[:])
                    pend.append((accb, (gli + 1, t),
                                 AF.Relu if li < 3 else AF.Copy))
                    if len(pend) > 2:
                        flush_pending()
                else:
                    if gsum is not None:
                        nc.vector.tensor_add(acc[:], acc[:], gsum[:])
                    nc.sync.dma_start(out[t * P:(t + 1) * P, :], acc[:])

        while pend:
            flush_pending()

    nc.compile()
    return nc


_CACHE = {}


def _get_nc(key):
    if key not in _CACHE:
        _CACHE[key] = _build(key[0], key[1])
    return _CACHE[key]


def _to_md(a):
    import ml_dtypes

    return np.ascontiguousarray(np.asarray(a, dtype=np.float32).astype(
        ml_dtypes.bfloat16))


def kernel(**inputs):
    x = np.ascontiguousarray(np.asarray(inputs["x"], dtype=np.float32))
    gate_w = _to_md(inputs["gate_w"])
    gate_b = np.ascontiguousarray(np.asarray(inputs["gate_b"], dtype=np.float32))
    wlist = [_to_md(inputs[n]) for n in W_NAMES]
    blist = [np.ascontiguousarray(np.asarray(inputs[n], dtype=np.float32)) for n in B_NAMES]

    has_gate_b = bool(np.any(gate_b))
    has_bias = tuple(bool(np.any(b)) for b in blist)
    nc = _get_nc((has_gate_b, has_bias))

    shared = {"gate_w": gate_w, "gate_b": gate_b}
    for n, w in zip(W_NAMES, wlist):
        shared[n] = w
    for n, b in zip(B_NAMES, blist):
        shared[n] = b

    core_ids = list(range(NCORES))
    in_maps = [dict(shared, x=x[i * R:(i + 1) * R]) for i in core_ids]
    res = run_bass_kernel_spmd(nc, in_maps, core_ids)
    return np.concatenate([res.results[i]["out"] for i in core_ids], axis=0)


if __name__ == "__main__":
    rng = np.random.default_rng(0)
    ins = {
        "x": rng.standard_normal((B, D), dtype=np.float32),
        "gate_w": rng.standard_normal((D, K), dtype=np.float32) * 0.02,
        "gate_b": np.zeros((K,), np.float32),
    }
    for n in W_NAMES:
        ins[n] = rng.standard_normal((K, D, D), dtype=np.float32) * 0.02
    for n in B_NAMES:
        ins[n] = np.zeros((K, D), np.float32)
    y = kernel(**ins)
    print("out", y.shape, y.dtype, float(np.abs(y).max()))


# revision 16
# speedup vs baseline: 1.1777x; 1.0837x over previous
"""Trainium2 Bass kernel for the BMoIE (dense mixture-of-experts) network.

Network (per sample):
    alpha = softmax(x @ gate_w + gate_b)                       # [B, 8]
    h = relu(sum_k alpha_k * (h @ w_l[k] + b_l[k]))            # 3 hidden blocks
    out = sum_k alpha_k * (h @ wo[k] + bo[k])                  # output block

Strategy: data-parallel over 8 NeuronCores (2048 rows each, SPMD, no
collectives). On each core, activations are kept feature-major ("hT",
[128 feat-partitions x batch]) so they can be the stationary matmul
operand; expert weights stream in natural layout as the moving operand,
producing batch-major per-expert outputs in PSUM. Matmul operands are
bf16 (full PE speed, half the HBM traffic of fp32; rel-l2 ~5e-3 vs the
2e-2 budget). The alpha-weighted sum over the 8 experts is split across
engines: experts 0,2,4,6,7 as a VectorE scalar_tensor_tensor chain
(fp32 accumulate) ending in a one-op tail, while experts 1,3,5 are
scaled on ScalarE and partial-summed by VectorE adds slotted mid-chain
(GpSimd is unused — its Q7 launch latency measured as an HW-only
stall). The combined result is
written as bf16 (the next layer's matmul input is bf16 anyway), so the
PE re-transposes to feature-major run at 1 cycle/row; each tile's
transposes are deferred two tiles behind the GEMMs so the in-order PE
never stalls on the combine chain. Weights load as one DMA per
(layer, expert) for few, large transfers.
"""

import sys

sys.path.insert(0, "/opt/trn_rl_repo")

import numpy as np

import concourse.bass as bass
import concourse.mybir as mybir
import concourse.tile as tile
from concourse import bacc
from concourse.bass_utils import run_bass_kernel_spmd
from concourse.masks import make_identity

P = 128           # partitions
D = 512           # model dim (= hidden dim)
K = 8             # experts
NCORES = 8
B = 16384
R = B // NCORES   # rows per core
NT = R // P       # 16 batch tiles per core
NCH = D // P      # 4 feature chunks
F32 = mybir.dt.float32
FR = mybir.dt.float32r
BF16 = mybir.dt.bfloat16
AF = mybir.ActivationFunctionType
ALU = mybir.AluOpType

W_NAMES = ("w0", "w1", "w2", "wo")
B_NAMES = ("b0", "b1", "b2", "bo")


def _build(has_gate_b, has_bias, w_bufs=None, ht_bufs=None, z_bufs=7, repeat=1,
           mode="full", gp_offload=3, bf16=True, split=True):
    """Trace + compile the per-core kernel. has_bias is a 4-tuple of bools.

    repeat>1 runs the whole 4-layer stack that many times (same weights,
    full DMA traffic each time) — used only for timing measurements.
    mode: "full" | "pe_only" (no combine/evict) — ablation variants for
    timing only (wrong results except "full").
    gp_offload: number of experts whose scale runs on ScalarE with the
    partial sum on GpSimd (reduces VectorE op count).
    """
    from contextlib import ExitStack

    MD = BF16 if bf16 else FR  # matmul operand dtype
    SD = MD                    # stationary (activations) dtype
    if w_bufs is None:
        w_bufs = 20 if bf16 else 12
    if ht_bufs is None:
        ht_bufs = 24 if bf16 else 20

    nc = bacc.Bacc("TRN2", target_bir_lowering=False, num_devices=NCORES)
    x = nc.dram_tensor("x", [R, D], F32, kind="ExternalInput")
    gate_w = nc.dram_tensor("gate_w", [D, K], MD, kind="ExternalInput")
    gate_b = nc.dram_tensor("gate_b", [K], F32, kind="ExternalInput")
    ws = [nc.dram_tensor(n, [K, D, D], MD, kind="ExternalInput") for n in W_NAMES]
    bs = [nc.dram_tensor(n, [K, D], FR, kind="ExternalInput") for n in B_NAMES]
    out = nc.dram_tensor("out", [R, D], F32, kind="ExternalOutput")

    any_bias = any(has_bias)

    with tile.TileContext(nc) as tc, ExitStack() as ctx:
        cst = ctx.enter_context(tc.tile_pool(name="cst", bufs=1))
        wpool = ctx.enter_context(tc.tile_pool(name="wpool", bufs=w_bufs))
        htp = ctx.enter_context(tc.tile_pool(name="htp", bufs=ht_bufs))
        accp = ctx.enter_context(tc.tile_pool(name="accp", bufs=3))
        accbp = ctx.enter_context(tc.tile_pool(name="accbp", bufs=4))
        xbp = ctx.enter_context(tc.tile_pool(name="xbp", bufs=3))
        smp = ctx.enter_context(tc.tile_pool(name="smp", bufs=4))
        zp = ctx.enter_context(tc.tile_pool(name="zp", bufs=z_bufs, space="PSUM"))
        trp = ctx.enter_context(tc.tile_pool(name="trp", bufs=8 - z_bufs, space="PSUM"))

        ident = cst.tile([P, P], F32, tag="ident")
        make_identity(nc, ident[:])
        identb = cst.tile([P, P], SD, tag="identb")
        nc.scalar.activation(identb[:], ident[:], AF.Copy)

        # gate_w [512, 8] -> [128, 4*8] (chunk c at cols c*8..)
        gw = cst.tile([P, NCH * K], MD, tag="gw")
        for c in range(NCH):
            nc.sync.dma_start(gw[:, c * K:(c + 1) * K], gate_w[c * P:(c + 1) * P, :])

        gb_bc = None
        if has_gate_b:
            ones_row = cst.tile([1, P], F32, tag="ones_row")
            nc.vector.memset(ones_row[:], 1.0)
            gb_row = cst.tile([1, K], F32, tag="gb_row")
            nc.sync.dma_start(gb_row[:], gate_b[None, :])
            gb_ps = trp.tile([P, D], F32, tag="tr")
            nc.tensor.matmul(gb_ps[:, :K], ones_row[:], gb_row[:])
            gb_bc = cst.tile([P, K], F32, tag="gb_bc")
            nc.scalar.activation(gb_bc[:], gb_ps[:, :K], AF.Copy)

        bl_sb = [None] * 4
        if any_bias:
            for li in range(4):
                if has_bias[li]:
                    blt = cst.tile([K, D], FR, tag=f"bl{li}")
                    nc.sync.dma_start(blt[:], bs[li][:, :])
                    bl_sb[li] = blt
            alphaT = cst.tile([K, R], FR, tag="alphaT")

        alpha = cst.tile([P, NT * K], F32, tag="alpha")

        # ---- prologue: load x tiles, transpose to feature-major, gate ----
        hT = {}
        for t in range(NT):
            xb = xbp.tile([P, D], F32, tag="xb")
            nc.sync.dma_start(xb[:], x[t * P:(t + 1) * P, :])
            trt = trp.tile([P, NCH, P], F32, tag="tr")
            for c in range(NCH):
                nc.tensor.transpose(trt[:, c, :], xb[:, c * P:(c + 1) * P], ident[:])
            ht = htp.tile([P, NCH, P], SD, tag="ht")
            nc.scalar.activation(ht[:], trt[:], AF.Copy)
            hT[(0, t)] = ht

            # gate for this tile (batch-major logits)
            lg = zp.tile([P, D], F32, tag="z")
            for c in range(NCH):
                nc.tensor.matmul(
                    lg[:, :K],
                    ht[:, c, :],
                    gw[:, c * K:(c + 1) * K],
                    start=(c == 0),
                    stop=(c == NCH - 1),
                )
            ex = smp.tile([P, K], F32, tag="ex")
            if has_gate_b:
                nc.vector.scalar_tensor_tensor(
                    ex[:], lg[:, :K], 1.0, gb_bc[:], op0=ALU.mult, op1=ALU.add
                )
                nc.scalar.activation(ex[:], ex[:], AF.Exp)
            else:
                nc.scalar.activation(ex[:], lg[:, :K], AF.Exp)
            ssum = smp.tile([P, 1], F32, tag="ssum")
            nc.vector.reduce_sum(ssum[:], ex[:], axis=mybir.AxisListType.X)
            rec = smp.tile([P, 1], F32, tag="rec")
            nc.vector.reciprocal(rec[:], ssum[:])
            nc.vector.tensor_scalar_mul(alpha[:, t * K:(t + 1) * K], ex[:], rec[:])

            if any_bias:
                # alphaT[:, t*128:(t+1)*128] = alpha_tile.T (8 x 128)
                at_ps = trp.tile([P, NCH, P], F32, tag="tr")
                nc.tensor.transpose(
                    at_ps[:K, 0, :], alpha[:, t * K:(t + 1) * K], ident[:]
                )
                nc.scalar.activation(
                    alphaT[:, t * P:(t + 1) * P], at_ps[:K, 0, :], AF.Copy
                )

        # ---- 4 MoIE blocks (x repeat for timing builds) ----
        # tile t's transposes are deferred until after tile t+1's GEMMs
        # (across layer boundaries too): the PE executes in order, so
        # transposing right after tile t's GEMMs would stall the PE on
        # the combine chain.
        pend = []

        def flush_pending():
            accb, dst, af = pend.pop(0)
            trt = trp.tile([P, NCH, P], SD, tag="tr")
            for c in range(NCH):
                nc.tensor.transpose(
                    trt[:, c, :], accb[:, c * P:(c + 1) * P], identb[:]
                )
            ht_n = htp.tile([P, NCH, P], SD, tag="ht")
            nc.scalar.activation(ht_n[:], trt[:], af)
            hT[dst] = ht_n

        for gli in range(4 * repeat):
            li = gli % 4
            last = gli == 4 * repeat - 1
            # stream this layer's weights (reused across all 16 batch tiles);
            # one big DMA per expert: [128, 4 chunks, 512]
            wsr = ws[li].rearrange("k (c p) o -> k p c o", p=P)
            wt = {}
            for k in range(K):
                w_t = wpool.tile([P, NCH, D], MD, tag="w", name=f"w_{gli}_{k}")
                nc.sync.dma_start(w_t[:], wsr[k])
                wt[k] = w_t

            for t in range(NT):
                bias_sb = None
                if has_bias[li]:
                    b_ps = trp.tile([P, D], F32, tag="tr")
                    nc.tensor.matmul(
                        b_ps[:],
                        alphaT[:, t * P:(t + 1) * P],
                        bl_sb[li][:],
                    )
                    bias_sb = smp.tile([P, D], F32, tag="bias_sb")
                    nc.scalar.activation(bias_sb[:], b_ps[:], AF.Copy)

                while (gli, t) not in hT:
                    flush_pending()
                ht_in = hT[(gli, t)]
                zs = []
                for k in range(K):
                    z = zp.tile([P, D], F32, tag="z", name=f"z_{gli}_{t}_{k}")
                    for c in range(NCH):
                        nc.tensor.matmul(
                            z[:],
                            ht_in[:, c, :],
                            wt[k][:, c, :],
                            start=(c == 0),
                            stop=(c == NCH - 1),
                        )
                    zs.append(z)

                if mode in ("pe_only",):
                    continue

                # weighted sum over experts: VectorE STT chain for the head
                # experts; tail experts scaled on ScalarE with partial sums
                # on GpSimd. Final add writes bf16 (except last layer, which
                # is DMAed out in fp32).
                # side chain on ScalarE+GpSimd takes EARLY experts (1,3,5)
                # so the GpSimd adds land mid-tile; the DVE STT chain takes
                # 0,2,4,6,7 and ends with a single add tail after z7 — the
                # deferred transposes then never wait on slow engines.
                side = (1, 3, 5)
                sk = {}
                for k in side:
                    a_ap = alpha[:, t * K + k:t * K + k + 1]
                    skt = smp.tile([P, D], F32, tag="sk", bufs=4)
                    nc.scalar.activation(skt[:], zs[k][:], AF.Copy, scale=a_ap)
                    sk[k] = skt
                # main STT chain on DVE over 0,2,4,6,7; the side-chain adds
                # interleave into the DVE stream mid-tile (all-SBUF operands
                # run at 2x), keeping GpSimd out entirely — its Q7 launch
                # latency showed up as an HW-only stall in earlier variants
                acc = accp.tile([P, D], F32, tag="acc")
                gsum = None
                main = [k for k in range(K) if k not in side]
                for j, k in enumerate(main):
                    a_ap = alpha[:, t * K + k:t * K + k + 1]
                    if j == 0:
                        if bias_sb is not None:
                            nc.vector.scalar_tensor_tensor(
                                acc[:], zs[k][:], a_ap, bias_sb[:],
                                op0=ALU.mult, op1=ALU.add,
                            )
                        else:
                            nc.vector.tensor_scalar_mul(acc[:], zs[k][:], a_ap)
                    else:
                        nc.vector.scalar_tensor_tensor(
                            acc[:], zs[k][:], a_ap, acc[:],
                            op0=ALU.mult, op1=ALU.add,
                        )
                    if j == 2:
                        gs2 = smp.tile([P, D], F32, tag="gs", bufs=2)
                        nc.vector.tensor_add(gs2[:], sk[1][:], sk[3][:])
                        gsum = gs2
                    elif j == 3:
                        gs3 = smp.tile([P, D], F32, tag="gs3", bufs=2)
                        nc.vector.tensor_add(gs3[:], gsum[:], sk[5][:])
                        gsum = gs3

                if not last:
                    # final combine add writes bf16 directly (no extra
                    # rounding vs the bf16 matmul input); relu fuses into
                    # the PSUM->SBUF eviction copy after the transposes
                    accb = accbp.tile([P, D], SD, tag="accb")
                    if gsum is not None:
                        nc.vector.tensor_add(accb[:], acc[:], gsum[:])
                    else:
                        nc.vector.tensor_copy(accb[:], acc[:])
                    pend.append((accb, (gli + 1, t),
                                 AF.Relu if li < 3 else AF.Copy))
                    if len(pend) > 2:
                        flush_pending()
                else:
                    if gsum is not None:
                        nc.vector.tensor_add(acc[:], acc[:], gsum[:])
                    nc.sync.dma_start(out[t * P:(t + 1) * P, :], acc[:])

        while pend:
            flush_pending()

    nc.compile()
    return nc


_CACHE = {}


def _get_nc(key):
    if key not in _CACHE:
        _CACHE[key] = _build(key[0], key[1])
    return _CACHE[key]


def _to_md(a):
    import ml_dtypes

    return np.ascontiguousarray(np.asarray(a, dtype=np.float32).astype(
        ml_dtypes.bfloat16))


def kernel(**inputs):
    x = np.ascontiguousarray(np.asarray(inputs["x"], dtype=np.float32))
    gate_w = _to_md(inputs["gate_w"])
    gate_b = np.ascontiguousarray(np.asarray(inputs["gate_b"], dtype=np.float32))
    wlist = [_to_md(inputs[n]) for n in W_NAMES]
    blist = [np.ascontiguousarray(np.asarray(inputs[n], dtype=np.float32)) for n in B_NAMES]

    has_gate_b = bool(np.any(gate_b))
    has_bias = tuple(bool(np.any(b)) for b in blist)
    nc = _get_nc((has_gate_b, has_bias))

    shared = {"gate_w": gate_w, "gate_b": gate_b}
    for n, w in zip(W_NAMES, wlist):
        shared[n] = w
    for n, b in zip(B_NAMES, blist):
        shared[n] = b

    core_ids = list(range(NCORES))
    in_maps = [dict(shared, x=x[i * R:(i + 1) * R]) for i in core_ids]
    res = run_bass_kernel_spmd(nc, in_maps, core_ids)
    return np.concatenate([res.results[i]["out"] for i in core_ids], axis=0)


if __name__ == "__main__":
    rng = np.random.default_rng(0)
    ins = {
        "x": rng.standard_normal((B, D), dtype=np.float32),
        "gate_w": rng.standard_normal((D, K), dtype=np.float32) * 0.02,
        "gate_b": np.zeros((K,), np.float32),
    }
    for n in W_NAMES:
        ins[n] = rng.standard_normal((K, D, D), dtype=np.float32) * 0.02
    for n in B_NAMES:
        ins[n] = np.zeros((K, D), np.float32)
    y = kernel(**ins)
    print("out", y.shape, y.dtype, float(np.abs(y).max()))


# revision 17
# speedup vs baseline: 1.2024x; 1.0210x over previous
"""Trainium2 Bass kernel for the BMoIE (dense mixture-of-experts) network.

Network (per sample):
    alpha = softmax(x @ gate_w + gate_b)                       # [B, 8]
    h = relu(sum_k alpha_k * (h @ w_l[k] + b_l[k]))            # 3 hidden blocks
    out = sum_k alpha_k * (h @ wo[k] + bo[k])                  # output block

Strategy: data-parallel over 8 NeuronCores (2048 rows each, SPMD, no
collectives). On each core, activations are kept feature-major ("hT",
[128 feat-partitions x batch]) so they can be the stationary matmul
operand; expert weights stream in natural layout as the moving operand,
producing batch-major per-expert outputs in PSUM. Matmul operands are
bf16 (full PE speed, half the HBM traffic of fp32; rel-l2 ~5e-3 vs the
2e-2 budget). The alpha-weighted sum over the 8 experts is split across
engines: experts 0,2,4,6,7 as a VectorE scalar_tensor_tensor chain
(fp32 accumulate) ending in a one-op tail, while experts 1,3,5 are
scaled on ScalarE and partial-summed by a VectorE add slotted
mid-chain (GpSimd is unused — its Q7 launch latency measured as an
HW-only stall). The combined result is
written as bf16 (the next layer's matmul input is bf16 anyway), so the
PE re-transposes to feature-major run at 1 cycle/row; each tile's
transposes are deferred two tiles behind the GEMMs so the in-order PE
never stalls on the combine chain. Weights load as one DMA per
(layer, expert) for few, large transfers.
"""

import sys

sys.path.insert(0, "/opt/trn_rl_repo")

import numpy as np

import concourse.bass as bass
import concourse.mybir as mybir
import concourse.tile as tile
from concourse import bacc
from concourse.bass_utils import run_bass_kernel_spmd
from concourse.masks import make_identity

P = 128           # partitions
D = 512           # model dim (= hidden dim)
K = 8             # experts
NCORES = 8
B = 16384
R = B // NCORES   # rows per core
NT = R // P       # 16 batch tiles per core
NCH = D // P      # 4 feature chunks
F32 = mybir.dt.float32
FR = mybir.dt.float32r
BF16 = mybir.dt.bfloat16
AF = mybir.ActivationFunctionType
ALU = mybir.AluOpType

W_NAMES = ("w0", "w1", "w2", "wo")
B_NAMES = ("b0", "b1", "b2", "bo")


def _build(has_gate_b, has_bias, w_bufs=None, ht_bufs=None, z_bufs=7, repeat=1,
           mode="full", gp_offload=3, bf16=True, split=True):
    """Trace + compile the per-core kernel. has_bias is a 4-tuple of bools.

    repeat>1 runs the whole 4-layer stack that many times (same weights,
    full DMA traffic each time) — used only for timing measurements.
    mode: "full" | "pe_only" (no combine/evict) — ablation variants for
    timing only (wrong results except "full").
    gp_offload: number of experts whose scale runs on ScalarE with the
    partial sum on GpSimd (reduces VectorE op count).
    """
    from contextlib import ExitStack

    MD = BF16 if bf16 else FR  # matmul operand dtype
    SD = MD                    # stationary (activations) dtype
    if w_bufs is None:
        w_bufs = 24 if bf16 else 12
    if ht_bufs is None:
        ht_bufs = 28 if bf16 else 20

    nc = bacc.Bacc("TRN2", target_bir_lowering=False, num_devices=NCORES)
    x = nc.dram_tensor("x", [R, D], F32, kind="ExternalInput")
    gate_w = nc.dram_tensor("gate_w", [D, K], MD, kind="ExternalInput")
    gate_b = nc.dram_tensor("gate_b", [K], F32, kind="ExternalInput")
    ws = [nc.dram_tensor(n, [K, D, D], MD, kind="ExternalInput") for n in W_NAMES]
    bs = [nc.dram_tensor(n, [K, D], FR, kind="ExternalInput") for n in B_NAMES]
    out = nc.dram_tensor("out", [R, D], F32, kind="ExternalOutput")

    any_bias = any(has_bias)

    with tile.TileContext(nc) as tc, ExitStack() as ctx:
        cst = ctx.enter_context(tc.tile_pool(name="cst", bufs=1))
        wpool = ctx.enter_context(tc.tile_pool(name="wpool", bufs=w_bufs))
        htp = ctx.enter_context(tc.tile_pool(name="htp", bufs=ht_bufs))
        accp = ctx.enter_context(tc.tile_pool(name="accp", bufs=3))
        accbp = ctx.enter_context(tc.tile_pool(name="accbp", bufs=5))
        xbp = ctx.enter_context(tc.tile_pool(name="xbp", bufs=3))
        smp = ctx.enter_context(tc.tile_pool(name="smp", bufs=4))
        zp = ctx.enter_context(tc.tile_pool(name="zp", bufs=z_bufs, space="PSUM"))
        trp = ctx.enter_context(tc.tile_pool(name="trp", bufs=8 - z_bufs, space="PSUM"))

        ident = cst.tile([P, P], F32, tag="ident")
        make_identity(nc, ident[:])
        identb = cst.tile([P, P], SD, tag="identb")
        nc.scalar.activation(identb[:], ident[:], AF.Copy)

        # gate_w [512, 8] -> [128, 4*8] (chunk c at cols c*8..)
        gw = cst.tile([P, NCH * K], MD, tag="gw")
        for c in range(NCH):
            nc.sync.dma_start(gw[:, c * K:(c + 1) * K], gate_w[c * P:(c + 1) * P, :])

        gb_bc = None
        if has_gate_b:
            ones_row = cst.tile([1, P], F32, tag="ones_row")
            nc.vector.memset(ones_row[:], 1.0)
            gb_row = cst.tile([1, K], F32, tag="gb_row")
            nc.sync.dma_start(gb_row[:], gate_b[None, :])
            gb_ps = trp.tile([P, D], F32, tag="tr")
            nc.tensor.matmul(gb_ps[:, :K], ones_row[:], gb_row[:])
            gb_bc = cst.tile([P, K], F32, tag="gb_bc")
            nc.scalar.activation(gb_bc[:], gb_ps[:, :K], AF.Copy)

        bl_sb = [None] * 4
        if any_bias:
            for li in range(4):
                if has_bias[li]:
                    blt = cst.tile([K, D], FR, tag=f"bl{li}")
                    nc.sync.dma_start(blt[:], bs[li][:, :])
                    bl_sb[li] = blt
            alphaT = cst.tile([K, R], FR, tag="alphaT")

        alpha = cst.tile([P, NT * K], F32, tag="alpha")

        # ---- prologue: load x tiles, transpose to feature-major, gate ----
        hT = {}
        for t in range(NT):
            xb = xbp.tile([P, D], F32, tag="xb")
            nc.sync.dma_start(xb[:], x[t * P:(t + 1) * P, :])
            trt = trp.tile([P, NCH, P], F32, tag="tr")
            for c in range(NCH):
                nc.tensor.transpose(trt[:, c, :], xb[:, c * P:(c + 1) * P], ident[:])
            ht = htp.tile([P, NCH, P], SD, tag="ht")
            nc.scalar.activation(ht[:], trt[:], AF.Copy)
            hT[(0, t)] = ht

            # gate for this tile (batch-major logits)
            lg = zp.tile([P, D], F32, tag="z")
            for c in range(NCH):
                nc.tensor.matmul(
                    lg[:, :K],
                    ht[:, c, :],
                    gw[:, c * K:(c + 1) * K],
                    start=(c == 0),
                    stop=(c == NCH - 1),
                )
            ex = smp.tile([P, K], F32, tag="ex")
            if has_gate_b:
                nc.vector.scalar_tensor_tensor(
                    ex[:], lg[:, :K], 1.0, gb_bc[:], op0=ALU.mult, op1=ALU.add
                )
                nc.scalar.activation(ex[:], ex[:], AF.Exp)
            else:
                nc.scalar.activation(ex[:], lg[:, :K], AF.Exp)
            ssum = smp.tile([P, 1], F32, tag="ssum")
            nc.vector.reduce_sum(ssum[:], ex[:], axis=mybir.AxisListType.X)
            rec = smp.tile([P, 1], F32, tag="rec")
            nc.vector.reciprocal(rec[:], ssum[:])
            nc.vector.tensor_scalar_mul(alpha[:, t * K:(t + 1) * K], ex[:], rec[:])

            if any_bias:
                # alphaT[:, t*128:(t+1)*128] = alpha_tile.T (8 x 128)
                at_ps = trp.tile([P, NCH, P], F32, tag="tr")
                nc.tensor.transpose(
                    at_ps[:K, 0, :], alpha[:, t * K:(t + 1) * K], ident[:]
                )
                nc.scalar.activation(
                    alphaT[:, t * P:(t + 1) * P], at_ps[:K, 0, :], AF.Copy
                )

        # ---- 4 MoIE blocks (x repeat for timing builds) ----
        # tile t's transposes are deferred until after tile t+1's GEMMs
        # (across layer boundaries too): the PE executes in order, so
        # transposing right after tile t's GEMMs would stall the PE on
        # the combine chain.
        pend = []

        def flush_pending():
            accb, dst, af = pend.pop(0)
            trt = trp.tile([P, NCH, P], SD, tag="tr")
            for c in range(NCH):
                nc.tensor.transpose(
                    trt[:, c, :], accb[:, c * P:(c + 1) * P], identb[:]
                )
            ht_n = htp.tile([P, NCH, P], SD, tag="ht")
            nc.scalar.activation(ht_n[:], trt[:], af)
            hT[dst] = ht_n

        for gli in range(4 * repeat):
            li = gli % 4
            last = gli == 4 * repeat - 1
            # stream this layer's weights (reused across all 16 batch tiles);
            # one big DMA per expert: [128, 4 chunks, 512]
            wsr = ws[li].rearrange("k (c p) o -> k p c o", p=P)
            wt = {}
            for k in range(K):
                w_t = wpool.tile([P, NCH, D], MD, tag="w", name=f"w_{gli}_{k}")
                nc.sync.dma_start(w_t[:], wsr[k])
                wt[k] = w_t

            for t in range(NT):
                bias_sb = None
                if has_bias[li]:
                    b_ps = trp.tile([P, D], F32, tag="tr")
                    nc.tensor.matmul(
                        b_ps[:],
                        alphaT[:, t * P:(t + 1) * P],
                        bl_sb[li][:],
                    )
                    bias_sb = smp.tile([P, D], F32, tag="bias_sb")
                    nc.scalar.activation(bias_sb[:], b_ps[:], AF.Copy)

                while (gli, t) not in hT:
                    flush_pending()
                ht_in = hT[(gli, t)]
                zs = []
                for k in range(K):
                    z = zp.tile([P, D], F32, tag="z", name=f"z_{gli}_{t}_{k}")
                    for c in range(NCH):
                        nc.tensor.matmul(
                            z[:],
                            ht_in[:, c, :],
                            wt[k][:, c, :],
                            start=(c == 0),
                            stop=(c == NCH - 1),
                        )
                    zs.append(z)

                if mode in ("pe_only",):
                    continue

                # weighted sum over experts: VectorE STT chain for the head
                # experts; tail experts scaled on ScalarE with partial sums
                # on GpSimd. Final add writes bf16 (except last layer, which
                # is DMAed out in fp32).
                # side chain on ScalarE+GpSimd takes EARLY experts (1,3,5)
                # so the GpSimd adds land mid-tile; the DVE STT chain takes
                # 0,2,4,6,7 and ends with a single add tail after z7 — the
                # deferred transposes then never wait on slow engines.
                side = (1, 3)
                sk = {}
                for k in side:
                    a_ap = alpha[:, t * K + k:t * K + k + 1]
                    skt = smp.tile([P, D], F32, tag="sk", bufs=4)
                    nc.scalar.activation(skt[:], zs[k][:], AF.Copy, scale=a_ap)
                    sk[k] = skt
                # main STT chain on DVE over 0,2,4,6,7; the side-chain adds
                # interleave into the DVE stream mid-tile (all-SBUF operands
                # run at 2x), keeping GpSimd out entirely — its Q7 launch
                # latency showed up as an HW-only stall in earlier variants
                acc = accp.tile([P, D], F32, tag="acc")
                gsum = None
                main = [k for k in range(K) if k not in side]
                for j, k in enumerate(main):
                    a_ap = alpha[:, t * K + k:t * K + k + 1]
                    if j == 0:
                        if bias_sb is not None:
                            nc.vector.scalar_tensor_tensor(
                                acc[:], zs[k][:], a_ap, bias_sb[:],
                                op0=ALU.mult, op1=ALU.add,
                            )
                        else:
                            nc.vector.tensor_scalar_mul(acc[:], zs[k][:], a_ap)
                    else:
                        nc.vector.scalar_tensor_tensor(
                            acc[:], zs[k][:], a_ap, acc[:],
                            op0=ALU.mult, op1=ALU.add,
                        )
                    if j == 2:
                        gs2 = smp.tile([P, D], F32, tag="gs", bufs=2)
                        nc.vector.tensor_add(gs2[:], sk[1][:], sk[3][:])
                        gsum = gs2

                if not last:
                    # final combine add writes bf16 directly (no extra
                    # rounding vs the bf16 matmul input); relu fuses into
                    # the PSUM->SBUF eviction copy after the transposes
                    accb = accbp.tile([P, D], SD, tag="accb")
                    if gsum is not None:
                        nc.vector.tensor_add(accb[:], acc[:], gsum[:])
                    else:
                        nc.vector.tensor_copy(accb[:], acc[:])
                    pend.append((accb, (gli + 1, t),
                                 AF.Relu if li < 3 else AF.Copy))
                    if len(pend) > 3:
                        flush_pending()
                else:
                    if gsum is not None:
                        nc.vector.tensor_add(acc[:], acc[:], gsum[:])
                    nc.sync.dma_start(out[t * P:(t + 1) * P, :], acc[:])

        while pend:
            flush_pending()

    nc.compile()
    return nc


_CACHE = {}


def _get_nc(key):
    if key not in _CACHE:
        _CACHE[key] = _build(key[0], key[1])
    return _CACHE[key]


def _to_md(a):
    import ml_dtypes

    return np.ascontiguousarray(np.asarray(a, dtype=np.float32).astype(
        ml_dtypes.bfloat16))


def kernel(**inputs):
    x = np.ascontiguousarray(np.asarray(inputs["x"], dtype=np.float32))
    gate_w = _to_md(inputs["gate_w"])
    gate_b = np.ascontiguousarray(np.asarray(inputs["gate_b"], dtype=np.float32))
    wlist = [_to_md(inputs[n]) for n in W_NAMES]
    blist = [np.ascontiguousarray(np.asarray(inputs[n], dtype=np.float32)) for n in B_NAMES]

    has_gate_b = bool(np.any(gate_b))
    has_bias = tuple(bool(np.any(b)) for b in blist)
    nc = _get_nc((has_gate_b, has_bias))

    shared = {"gate_w": gate_w, "gate_b": gate_b}
    for n, w in zip(W_NAMES, wlist):
        shared[n] = w
    for n, b in zip(B_NAMES, blist):
        shared[n] = b

    core_ids = list(range(NCORES))
    in_maps = [dict(shared, x=x[i * R:(i + 1) * R]) for i in core_ids]
    res = run_bass_kernel_spmd(nc, in_maps, core_ids)
    return np.concatenate([res.results[i]["out"] for i in core_ids], axis=0)


if __name__ == "__main__":
    rng = np.random.default_rng(0)
    ins = {
        "x": rng.standard_normal((B, D), dtype=np.float32),
        "gate_w": rng.standard_normal((D, K), dtype=np.float32) * 0.02,
        "gate_b": np.zeros((K,), np.float32),
    }
    for n in W_NAMES:
        ins[n] = rng.standard_normal((K, D, D), dtype=np.float32) * 0.02
    for n in B_NAMES:
        ins[n] = np.zeros((K, D), np.float32)
    y = kernel(**ins)
    print("out", y.shape, y.dtype, float(np.abs(y).max()))
